# revision 32
# baseline (speedup 1.0000x reference)
"""Trainium2 Bass kernel: 2-layer GAT on 500 disjoint 200-node chain graphs.

Chain topology => in-neighborhood of node i is {i-1, i, i+1} (clipped at
chain ends) => segment-softmax attention becomes a 3-point stencil. The
aggregation sum_j alpha_j h_j is computed as 3 PSUM-accumulated matmuls
over alpha-prescaled (and free-dim-shifted) copies of the matmul input,
entirely in channel-major layout. Softmax math runs batched in
chain-major [126, 200] layout. 8 cores x 63 chains (500 real + 4 pad).
"""
import sys
sys.path.insert(0, '/opt/trn_rl_repo')
import numpy as np
import ml_dtypes
from contextlib import ExitStack

import concourse.bass as bass
import concourse.bacc as bacc
import concourse.mybir as mybir
from concourse import tile
from concourse.bass_utils import run_bass_kernel_spmd

F32 = mybir.dt.float32
BF16 = mybir.dt.bfloat16
BF = ml_dtypes.bfloat16
AF = mybir.ActivationFunctionType
OP = mybir.AluOpType

G = 63
L = 200
N = G * L          # 12600
D = 64
HID = 128
NEG = 0.2
NEG_BIG = -1e30

ATC = [(i * 512, 512) for i in range(24)] + [(12288, 312)]
SUP = [(i * 1024, 1024) for i in range(12)] + [(12288, 312)]     # psum supertiles
def inner(g0, gw):
    return [(g0, min(512, gw))] + ([(g0 + 512, gw - 512)] if gw > 512 else [])
def inner2(g0, gw):
    out = []
    while gw > 0:
        w = min(512, gw)
        out.append((g0, w))
        g0 += w; gw -= w
    return out

_cache = {}


def build_nc():
    nc = bacc.Bacc("TRN2", target_bir_lowering=False, debug=False)

    xT2 = nc.dram_tensor("xT2", [2 * D, N], BF16, kind="ExternalInput")
    waux1 = nc.dram_tensor("waux1", [D, 4], BF16, kind="ExternalInput")
    w1blk = nc.dram_tensor("w1blk", [2 * D, 2 * HID], BF16, kind="ExternalInput")
    w2 = nc.dram_tensor("w2", [2 * HID, HID], BF16, kind="ExternalInput")
    attv2 = nc.dram_tensor("attv2", [HID, 2], BF16, kind="ExternalInput")
    waux2 = nc.dram_tensor("waux2", [2 * HID, 2], BF16, kind="ExternalInput")
    ident = nc.dram_tensor("ident", [HID, HID], BF16, kind="ExternalInput")
    out = nc.dram_tensor("out", [HID, N], F32, kind="ExternalOutput")

    with ExitStack() as ctx:
        tc = ctx.enter_context(tile.TileContext(nc))
        const = ctx.enter_context(tc.tile_pool(name="const", bufs=1))
        big = ctx.enter_context(tc.tile_pool(name="big", bufs=1))
        rows = ctx.enter_context(tc.tile_pool(name="rows", bufs=1))
        cmp_ = ctx.enter_context(tc.tile_pool(name="cmp", bufs=1))
        xch = ctx.enter_context(tc.tile_pool(name="xch", bufs=2))
        och = ctx.enter_context(tc.tile_pool(name="och", bufs=2))
        psA = ctx.enter_context(tc.tile_pool(name="psA", bufs=1, space="PSUM"))
        psB = ctx.enter_context(tc.tile_pool(name="psB", bufs=1, space="PSUM"))
        psC = ctx.enter_context(tc.tile_pool(name="psC", bufs=4, space="PSUM"))

        t_x = big.tile([2 * D, N], BF16, tag="xh")        # x.T stacked twice
        nc.sync.dma_start(t_x[:], xT2[:])
        t_w1blk = const.tile([2 * D, 2 * HID], BF16, tag="w1blk")
        nc.sync.dma_start(t_w1blk[:], w1blk[:])
        t_waux1 = const.tile([D, 4], BF16, tag="waux1")
        nc.sync.dma_start(t_waux1[:], waux1[:])
        t_w2a = const.tile([HID, HID], BF16, tag="w2a")
        t_w2b = const.tile([HID, HID], BF16, tag="w2b")
        nc.sync.dma_start(t_w2a[:], w2[0:HID, :])
        nc.sync.dma_start(t_w2b[:], w2[HID:2 * HID, :])
        t_waux2a = const.tile([HID, 2], BF16, tag="waux2a")
        t_waux2b = const.tile([HID, 2], BF16, tag="waux2b")
        nc.sync.dma_start(t_waux2a[:], waux2[0:HID, :])
        nc.sync.dma_start(t_waux2b[:], waux2[HID:2 * HID, :])
        t_I = const.tile([HID, HID], BF16, tag="ident")
        nc.sync.dma_start(t_I[:], ident[:])

        # bf16 beta row scratch in SBUF; f32 logit rows bounce via internal DRAM
        t_brow = rows.tile([4, N], BF16, tag="browsbf")
        d_at = nc.dram_tensor("scratch_at", [4, N], F32)

        # ============ LAYER 1 ============
        for i, (c0, cw) in enumerate(ATC):
            p_at = psC.tile([4, 512], F32, tag="atps")
            nc.tensor.matmul(p_at[:, :cw], t_waux1[:], t_x[0:D, c0:c0 + cw],
                             start=True, stop=True)
            st = och.tile([4, 512], F32, tag="atstage")
            if i % 2 == 0:
                nc.vector.tensor_copy(st[:, :cw], p_at[:, :cw])
            else:
                nc.scalar.copy(st[:, :cw], p_at[:, :cw])
            nc.sync.dma_start(d_at[:, c0:c0 + cw], st[:, :cw])

        # chain-major [126, 200] f32: head0 parts 0:63, head1 parts 63:126
        cm_s = cmp_.tile([126, L], F32, tag="cm_s")
        cm_d = cmp_.tile([126, L], F32, tag="cm_d")
        nc.sync.dma_start(cm_s[0:G, :], d_at[0:1, :])
        nc.sync.dma_start(cm_s[G:2 * G, :], d_at[1:2, :])
        nc.sync.dma_start(cm_d[0:G, :], d_at[2:3, :])
        nc.sync.dma_start(cm_d[G:2 * G, :], d_at[3:4, :])

        def chain_softmax(P, a_s, a_d):
            """alpha_p/s/n [P, L+2] f32 (zero guards at cols 0 and L+1, data 1..L)."""
            l_s = cmp_.tile([P, L + 2], F32, tag="l_s")
            l_p = cmp_.tile([P, L + 2], F32, tag="l_p")
            l_n = cmp_.tile([P, L + 2], F32, tag="l_n")
            d = slice(1, L + 1)
            nc.vector.tensor_add(l_s[:, d], a_s[:], a_d[:])
            nc.vector.scalar_tensor_tensor(l_s[:, d], l_s[:, d], NEG, l_s[:, d], OP.mult, OP.max)
            nc.vector.tensor_add(l_p[:, 2:L + 1], a_s[:, 0:L - 1], a_d[:, 1:L])
            nc.vector.scalar_tensor_tensor(l_p[:, 2:L + 1], l_p[:, 2:L + 1], NEG, l_p[:, 2:L + 1], OP.mult, OP.max)
            nc.vector.memset(l_p[:, 1:2], NEG_BIG)
            nc.vector.tensor_add(l_n[:, 1:L], a_s[:, 1:L], a_d[:, 0:L - 1])
            nc.vector.scalar_tensor_tensor(l_n[:, 1:L], l_n[:, 1:L], NEG, l_n[:, 1:L], OP.mult, OP.max)
            nc.vector.memset(l_n[:, L:L + 1], NEG_BIG)
            mx = cmp_.tile([P, L + 2], F32, tag="mx")
            nc.vector.tensor_max(mx[:, d], l_p[:, d], l_n[:, d])
            nc.vector.tensor_max(mx[:, d], mx[:, d], l_s[:, d])
            for lt in (l_s, l_p, l_n):
                nc.vector.tensor_sub(lt[:, d], lt[:, d], mx[:, d])
                nc.scalar.activation(lt[:, d], lt[:, d], AF.Exp)
            ssum = cmp_.tile([P, L + 2], F32, tag="ssum")
            nc.vector.tensor_add(ssum[:, d], l_s[:, d], l_p[:, d])
            nc.vector.tensor_add(ssum[:, d], ssum[:, d], l_n[:, d])
            nc.vector.tensor_scalar_add(ssum[:, d], ssum[:, d], 1e-16)
            rcp = cmp_.tile([P, L + 2], F32, tag="rcp")
            nc.vector.reciprocal(rcp[:, d], ssum[:, d])
            for lt in (l_s, l_p, l_n):
                nc.vector.tensor_mul(lt[:, d], lt[:, d], rcp[:, d])
                nc.vector.memset(lt[:, 0:1], 0.0)
                nc.vector.memset(lt[:, L + 1:L + 2], 0.0)
            return l_p, l_s, l_n

        a1_p, a1_s, a1_n = chain_softmax(126, cm_s, cm_d)

        # beta rows (source coords, bf16) into t_rows, then replicate into B tiles
        def make_B(alpha_cm, shift, nheads, Bt, bp, dup=False):
            """write shifted alpha rows for each head into t_brow[bp+h], then
            broadcast row -> Bt partitions [h*64..]. dup=True: single-head
            source replicated into one 64-partition tile."""
            if dup:
                nheads = 1
            for h in range(nheads):
                r = t_brow[bp + h:bp + h + 1, :]
                # shift=1: beta[j]=alpha[j+1] -> cols 2..201; 0: cols 1..200; -1: cols 0..199
                s0 = 1 + shift
                nc.gpsimd.dma_start(r, alpha_cm[h * G:(h + 1) * G, s0:s0 + L])
            PP = 128 // nheads
            for h in range(nheads):
                p0 = h * PP
                nc.sync.dma_start(Bt[p0:p0 + 1, :], t_brow[bp + h:bp + h + 1, :])
                k = 1
                while k < PP:
                    kk = min(k, PP - k)
                    nc.sync.dma_start(Bt[p0 + k:p0 + k + kk, :], Bt[p0:p0 + kk, :])
                    k += kk

        B_p = big.tile([128, N], BF16, tag="B_p")
        B_s = big.tile([128, N], BF16, tag="B_s")
        B_n = big.tile([128, N], BF16, tag="B_n")
        make_B(a1_p, 1, 2, B_p, 0)
        make_B(a1_s, 0, 2, B_s, 2)   # distinct scratch rows to avoid serialization
        make_B(a1_n, -1, 2, B_n, 0)

        g_hi_t = {}
        g_lo_t = {}
        for (s0, sw) in SUP:
            gh = big.tile([HID, 1024], BF16, tag=f"g_hi{s0}")
            gl = big.tile([HID, 1024], BF16, tag=f"g_lo{s0}")
            g_hi_t[s0] = gh
            g_lo_t[s0] = gl

        def prescale(dst, src_t, src_off, Bt, c0, cw):
            """dst[128, cw+2] = src[:, c0-1 : c0+cw+1] * B[...], with edge guards."""
            lo, hi = c0 - 1, c0 + cw + 1
            dlo = 0
            if lo < 0:
                nc.vector.memset(dst[:, 0:1], 0.0)
                lo, dlo = 0, 1
            if hi > N:
                nc.vector.memset(dst[:, cw + 1:cw + 2], 0.0)
                hi = N
            nc.vector.tensor_mul(dst[:, dlo:dlo + (hi - lo)],
                                 src_t[:, src_off + lo:src_off + hi], Bt[:, lo:hi])

        # L1 aggregation: per supertile group, per head
        for (g0, gw) in SUP:
            po_h0 = psA.tile([HID, 1024], F32, tag="ps_h0")
            po_h1 = psB.tile([HID, 1024], F32, tag="ps_h1")
            po = {0: po_h0, 1: po_h1}
            Xp = xch.tile([128, 1026], BF16, tag="Xp")
            Xs = xch.tile([128, 1026], BF16, tag="Xs")
            Xn = xch.tile([128, 1026], BF16, tag="Xn")
            prescale(Xp, t_x, 0, B_p, g0, gw)
            prescale(Xs, t_x, 0, B_s, g0, gw)
            prescale(Xn, t_x, 0, B_n, g0, gw)
            for (c0, cw) in inner(g0, gw):
                o = c0 - g0
                for h in (0, 1):
                    lhs = t_w1blk[:, h * HID:(h + 1) * HID]
                    nc.tensor.matmul(po[h][:, o:o + cw], lhs, Xp[:, o:o + cw], start=True, stop=False)
                    nc.tensor.matmul(po[h][:, o:o + cw], lhs, Xs[:, o + 1:o + 1 + cw], start=False, stop=False)
                    nc.tensor.matmul(po[h][:, o:o + cw], lhs, Xn[:, o + 2:o + 2 + cw], start=False, stop=True)
            nc.scalar.activation(g_hi_t[g0][:, :gw], po[0][:, :gw], AF.Gelu)
            nc.scalar.activation(g_lo_t[g0][:, :gw], po[1][:, :gw], AF.Gelu)
            for (c0, cw) in inner(g0, gw):
                o = c0 - g0
                p_at = psC.tile([4, 512], F32, tag="atps")
                nc.tensor.matmul(p_at[0:2, :cw], t_waux2a[:], g_hi_t[g0][:, o:o + cw],
                                 start=True, stop=False)
                nc.tensor.matmul(p_at[0:2, :cw], t_waux2b[:], g_lo_t[g0][:, o:o + cw],
                                 start=False, stop=True)
                st = och.tile([4, 512], F32, tag="atstage")
                nc.vector.tensor_copy(st[0:2, :cw], p_at[0:2, :cw])
                nc.sync.dma_start(d_at[0:2, c0:c0 + cw], st[0:2, :cw])

        # ============ LAYER 2 ============
        h2 = big.tile([HID, N + 2], BF16, tag="xh")   # reuse x slot; guards at 0 and N+1
        nc.vector.memset(h2[:, 0:1], 0.0)
        nc.vector.memset(h2[:, N + 1:N + 2], 0.0)
        for (g0, gw) in SUP:
            po = psA.tile([HID, 1024], F32, tag="ps_h0")
            for (c0, cw) in inner(g0, gw):
                o = c0 - g0
                nc.tensor.matmul(po[:, o:o + cw], t_w2a[:], g_hi_t[g0][:, o:o + cw], start=True, stop=False)
                nc.tensor.matmul(po[:, o:o + cw], t_w2b[:], g_lo_t[g0][:, o:o + cw], start=False, stop=True)
            nc.scalar.copy(h2[:, 1 + g0:1 + g0 + gw], po[:, :gw])

        cm2_s = cmp_.tile([G, L], F32, tag="cm_s")
        cm2_d = cmp_.tile([G, L], F32, tag="cm_d")
        nc.sync.dma_start(cm2_s[:, :], d_at[0:1, :])
        nc.sync.dma_start(cm2_d[:, :], d_at[1:2, :])
        a2_p, a2_s, a2_n = chain_softmax(G, cm2_s, cm2_d)

        B2_p = big.tile([128, N], BF16, tag="B_p")
        B2_s = big.tile([128, N], BF16, tag="B_s")
        B2_n = big.tile([128, N], BF16, tag="B_n")
        make_B(a2_p, 1, 1, B2_p, 0)
        make_B(a2_s, 0, 1, B2_s, 1)
        make_B(a2_n, -1, 1, B2_n, 2)

        def prescale2(dst, src_t, src_off, Bt, c0, cw):
            lo, hi = c0 - 1, c0 + cw + 1
            dlo = 0
            if lo < 0:
                nc.vector.memset(dst[:, 0:1], 0.0)
                lo, dlo = 0, 1
            if hi > N:
                nc.vector.memset(dst[:, cw + 1:cw + 2], 0.0)
                hi = N
            n = hi - lo
            nc.vector.tensor_mul(dst[0:64, dlo:dlo + n],
                                 src_t[0:64, src_off + lo:src_off + hi], Bt[:, lo:hi])
            nc.vector.tensor_mul(dst[64:128, dlo:dlo + n],
                                 src_t[64:128, src_off + lo:src_off + hi], Bt[:, lo:hi])

        for (g0, gw) in SUP:
            po = psB.tile([HID, 1024], F32, tag="ps_h1")
            Hp = xch.tile([128, 1026], BF16, tag="Xp")
            Hs = xch.tile([128, 1026], BF16, tag="Xs")
            Hn = xch.tile([128, 1026], BF16, tag="Xn")
            prescale(Hp, h2, 1, B2_p, g0, gw)
            prescale(Hs, h2, 1, B2_s, g0, gw)
            prescale(Hn, h2, 1, B2_n, g0, gw)
            for (c0, cw) in inner(g0, gw):
                o = c0 - g0
                nc.tensor.matmul(po[:, o:o + cw], t_I[:], Hp[:, o:o + cw], start=True, stop=False)
                nc.tensor.matmul(po[:, o:o + cw], t_I[:], Hs[:, o + 1:o + 1 + cw], start=False, stop=False)
                nc.tensor.matmul(po[:, o:o + cw], t_I[:], Hn[:, o + 2:o + 2 + cw], start=False, stop=True)
            t_out = och.tile([HID, 1024], F32, tag="outch")
            nc.scalar.activation(t_out[:, :gw], po[:, :gw], AF.Gelu)
            nc.sync.dma_start(out[:, g0:g0 + gw], t_out[:, :gw])

    nc.compile()
    return nc


def _prep(inputs):
    x = np.asarray(inputs["x"], np.float32)
    W1 = np.asarray(inputs["W1"], np.float32)
    att_src1 = np.asarray(inputs["att_src1"], np.float32)
    att_dst1 = np.asarray(inputs["att_dst1"], np.float32)
    W2 = np.asarray(inputs["W2"], np.float32)
    att_src2 = np.asarray(inputs["att_src2"], np.float32)
    att_dst2 = np.asarray(inputs["att_dst2"], np.float32)

    x_pad = np.zeros((8 * N, D), np.float32)
    x_pad[:x.shape[0]] = x

    waux1 = np.stack([W1[:, 0:HID] @ att_src1[0], W1[:, HID:2 * HID] @ att_src1[1],
                      W1[:, 0:HID] @ att_dst1[0], W1[:, HID:2 * HID] @ att_dst1[1]], axis=1)
    w1blk = np.zeros((2 * D, 2 * HID), np.float32)
    w1blk[0:D, 0:HID] = W1[:, 0:HID]
    w1blk[D:2 * D, HID:2 * HID] = W1[:, HID:2 * HID]
    attv2 = np.stack([att_src2[0], att_dst2[0]], axis=1)
    waux2 = np.stack([W2 @ att_src2[0], W2 @ att_dst2[0]], axis=1)

    common = dict(
        waux1=waux1.astype(BF), w1blk=w1blk.astype(BF), w2=W2.astype(BF),
        attv2=attv2.astype(BF), waux2=waux2.astype(BF), ident=np.eye(HID, dtype=np.float32).astype(BF),
    )
    in_maps = []
    for c in range(8):
        xT = np.ascontiguousarray(x_pad[c * N:(c + 1) * N].T)
        in_maps.append(dict(common, xT2=np.vstack([xT, xT]).astype(BF)))
    return in_maps, x.shape[0]


def kernel(**inputs):
    if "nc" not in _cache:
        _cache["nc"] = build_nc()
    in_maps, n_real = _prep(inputs)
    res = run_bass_kernel_spmd(_cache["nc"], in_maps, core_ids=list(range(8)))
    outs = [res.results[c]["out"] for c in range(8)]
    full = np.concatenate([o.T for o in outs], axis=0)
    return np.ascontiguousarray(full[:n_real]).astype(np.float32)


# revision 33
# speedup vs baseline: 1.0081x; 1.0081x over previous
"""Trainium2 Bass kernel: 2-layer GAT on 500 disjoint 200-node chain graphs.

Chain topology => in-neighborhood of node i is {i-1, i, i+1} (clipped at
chain ends) => segment-softmax attention becomes a 3-point stencil. The
aggregation sum_j alpha_j h_j is computed as 3 PSUM-accumulated matmuls
over alpha-prescaled (and free-dim-shifted) copies of the matmul input,
entirely in channel-major layout. Softmax math runs batched in
chain-major [126, 200] layout. 8 cores x 63 chains (500 real + 4 pad).
"""
import sys
sys.path.insert(0, '/opt/trn_rl_repo')
import numpy as np
import ml_dtypes
from contextlib import ExitStack

import concourse.bass as bass
import concourse.bacc as bacc
import concourse.mybir as mybir
from concourse import tile
from concourse.bass_utils import run_bass_kernel_spmd

F32 = mybir.dt.float32
BF16 = mybir.dt.bfloat16
BF = ml_dtypes.bfloat16
AF = mybir.ActivationFunctionType
OP = mybir.AluOpType

G = 63
L = 200
N = G * L          # 12600
D = 64
HID = 128
NEG = 0.2
NEG_BIG = -1e30

ATC = [(i * 512, 512) for i in range(24)] + [(12288, 312)]
SUP = [(i * 1024, 1024) for i in range(12)] + [(12288, 312)]     # psum supertiles
def inner(g0, gw):
    return [(g0, min(512, gw))] + ([(g0 + 512, gw - 512)] if gw > 512 else [])
def inner2(g0, gw):
    out = []
    while gw > 0:
        w = min(512, gw)
        out.append((g0, w))
        g0 += w; gw -= w
    return out

_cache = {}


def build_nc():
    nc = bacc.Bacc("TRN2", target_bir_lowering=False, debug=False)

    xT2 = nc.dram_tensor("xT2", [2 * D, N], BF16, kind="ExternalInput")
    waux1 = nc.dram_tensor("waux1", [D, 4], BF16, kind="ExternalInput")
    w1blk = nc.dram_tensor("w1blk", [2 * D, 2 * HID], BF16, kind="ExternalInput")
    w2 = nc.dram_tensor("w2", [2 * HID, HID], BF16, kind="ExternalInput")
    attv2 = nc.dram_tensor("attv2", [HID, 2], BF16, kind="ExternalInput")
    waux2 = nc.dram_tensor("waux2", [2 * HID, 2], BF16, kind="ExternalInput")
    ident = nc.dram_tensor("ident", [HID, HID], BF16, kind="ExternalInput")
    out = nc.dram_tensor("out", [HID, N], F32, kind="ExternalOutput")

    with ExitStack() as ctx:
        tc = ctx.enter_context(tile.TileContext(nc))
        const = ctx.enter_context(tc.tile_pool(name="const", bufs=1))
        big = ctx.enter_context(tc.tile_pool(name="big", bufs=1))
        rows = ctx.enter_context(tc.tile_pool(name="rows", bufs=1))
        cmp_ = ctx.enter_context(tc.tile_pool(name="cmp", bufs=1))
        xch = ctx.enter_context(tc.tile_pool(name="xch", bufs=2))
        och = ctx.enter_context(tc.tile_pool(name="och", bufs=2))
        psA = ctx.enter_context(tc.tile_pool(name="psA", bufs=1, space="PSUM"))
        psB = ctx.enter_context(tc.tile_pool(name="psB", bufs=1, space="PSUM"))
        psC = ctx.enter_context(tc.tile_pool(name="psC", bufs=4, space="PSUM"))

        t_x = big.tile([2 * D, N], BF16, tag="xh")        # x.T stacked twice
        nc.sync.dma_start(t_x[:], xT2[:])
        t_w1blk = const.tile([2 * D, 2 * HID], BF16, tag="w1blk")
        nc.sync.dma_start(t_w1blk[:], w1blk[:])
        t_waux1 = const.tile([D, 4], BF16, tag="waux1")
        nc.sync.dma_start(t_waux1[:], waux1[:])
        t_w2a = const.tile([HID, HID], BF16, tag="w2a")
        t_w2b = const.tile([HID, HID], BF16, tag="w2b")
        nc.sync.dma_start(t_w2a[:], w2[0:HID, :])
        nc.sync.dma_start(t_w2b[:], w2[HID:2 * HID, :])
        t_waux2a = const.tile([HID, 2], BF16, tag="waux2a")
        t_waux2b = const.tile([HID, 2], BF16, tag="waux2b")
        nc.sync.dma_start(t_waux2a[:], waux2[0:HID, :])
        nc.sync.dma_start(t_waux2b[:], waux2[HID:2 * HID, :])
        t_I = const.tile([HID, HID], BF16, tag="ident")
        nc.sync.dma_start(t_I[:], ident[:])

        d_at = nc.dram_tensor("scratch_at", [4, N], F32)
        d_brow = nc.dram_tensor("scratch_brow", [4, N], BF16)

        # ============ LAYER 1 ============
        for i, (c0, cw) in enumerate(ATC):
            p_at = psC.tile([4, 512], F32, tag="atps")
            nc.tensor.matmul(p_at[:, :cw], t_waux1[:], t_x[0:D, c0:c0 + cw],
                             start=True, stop=True)
            st = och.tile([4, 512], F32, tag="atstage")
            if i % 2 == 0:
                nc.vector.tensor_copy(st[:, :cw], p_at[:, :cw])
            else:
                nc.scalar.copy(st[:, :cw], p_at[:, :cw])
            nc.sync.dma_start(d_at[:, c0:c0 + cw], st[:, :cw])

        # chain-major [126, 200] f32: head0 parts 0:63, head1 parts 63:126
        cm_s = cmp_.tile([126, L], F32, tag="cm_s")
        cm_d = cmp_.tile([126, L], F32, tag="cm_d")
        nc.sync.dma_start(cm_s[0:G, :], d_at[0:1, :])
        nc.sync.dma_start(cm_s[G:2 * G, :], d_at[1:2, :])
        nc.sync.dma_start(cm_d[0:G, :], d_at[2:3, :])
        nc.sync.dma_start(cm_d[G:2 * G, :], d_at[3:4, :])

        def chain_softmax(P, a_s, a_d):
            """alpha_p/s/n [P, L+2] f32 (zero guards at cols 0 and L+1, data 1..L)."""
            l_s = cmp_.tile([P, L + 2], F32, tag="l_s")
            l_p = cmp_.tile([P, L + 2], F32, tag="l_p")
            l_n = cmp_.tile([P, L + 2], F32, tag="l_n")
            d = slice(1, L + 1)
            nc.vector.tensor_add(l_s[:, d], a_s[:], a_d[:])
            nc.vector.scalar_tensor_tensor(l_s[:, d], l_s[:, d], NEG, l_s[:, d], OP.mult, OP.max)
            nc.vector.tensor_add(l_p[:, 2:L + 1], a_s[:, 0:L - 1], a_d[:, 1:L])
            nc.vector.scalar_tensor_tensor(l_p[:, 2:L + 1], l_p[:, 2:L + 1], NEG, l_p[:, 2:L + 1], OP.mult, OP.max)
            nc.vector.memset(l_p[:, 1:2], NEG_BIG)
            nc.vector.tensor_add(l_n[:, 1:L], a_s[:, 1:L], a_d[:, 0:L - 1])
            nc.vector.scalar_tensor_tensor(l_n[:, 1:L], l_n[:, 1:L], NEG, l_n[:, 1:L], OP.mult, OP.max)
            nc.vector.memset(l_n[:, L:L + 1], NEG_BIG)
            mx = cmp_.tile([P, L + 2], F32, tag="mx")
            nc.vector.tensor_max(mx[:, d], l_p[:, d], l_n[:, d])
            nc.vector.tensor_max(mx[:, d], mx[:, d], l_s[:, d])
            for lt in (l_s, l_p, l_n):
                nc.vector.tensor_sub(lt[:, d], lt[:, d], mx[:, d])
                nc.scalar.activation(lt[:, d], lt[:, d], AF.Exp)
            ssum = cmp_.tile([P, L + 2], F32, tag="ssum")
            nc.vector.tensor_add(ssum[:, d], l_s[:, d], l_p[:, d])
            nc.vector.tensor_add(ssum[:, d], ssum[:, d], l_n[:, d])
            nc.vector.tensor_scalar_add(ssum[:, d], ssum[:, d], 1e-16)
            rcp = cmp_.tile([P, L + 2], F32, tag="rcp")
            nc.vector.reciprocal(rcp[:, d], ssum[:, d])
            for lt in (l_s, l_p, l_n):
                nc.vector.tensor_mul(lt[:, d], lt[:, d], rcp[:, d])
                nc.vector.memset(lt[:, 0:1], 0.0)
                nc.vector.memset(lt[:, L + 1:L + 2], 0.0)
            return l_p, l_s, l_n

        a1_p, a1_s, a1_n = chain_softmax(126, cm_s, cm_d)

        # beta rows (source coords, bf16) into t_rows, then replicate into B tiles
        def make_B(alpha_cm, shift, nheads, Bt, bp, dup=False):
            """write shifted alpha rows for each head into t_brow[bp+h], then
            broadcast row -> Bt partitions [h*64..]. dup=True: single-head
            source replicated into one 64-partition tile."""
            if dup:
                nheads = 1
            for h in range(nheads):
                # shift=1: beta[j]=alpha[j+1] -> cols 2..201; 0: cols 1..200; -1: cols 0..199
                s0 = 1 + shift
                nc.gpsimd.dma_start(d_brow[bp + h:bp + h + 1, :],
                                    alpha_cm[h * G:(h + 1) * G, s0:s0 + L])
            PP = 128 // nheads
            for h in range(nheads):
                p0 = h * PP
                nc.sync.dma_start(Bt[p0:p0 + PP, :],
                                  d_brow[bp + h:bp + h + 1, :].broadcast_to((PP, N)))

        B_p = big.tile([128, N], BF16, tag="B_p")
        B_s = big.tile([128, N], BF16, tag="B_s")
        B_n = big.tile([128, N], BF16, tag="B_n")
        make_B(a1_p, 1, 2, B_p, 0)
        make_B(a1_s, 0, 2, B_s, 2)   # distinct scratch rows to avoid serialization
        make_B(a1_n, -1, 2, B_n, 0)

        g_hi_t = {}
        g_lo_t = {}
        for (s0, sw) in SUP:
            gh = big.tile([HID, 1024], BF16, tag=f"g_hi{s0}")
            gl = big.tile([HID, 1024], BF16, tag=f"g_lo{s0}")
            g_hi_t[s0] = gh
            g_lo_t[s0] = gl

        def prescale(dst, src_t, src_off, Bt, c0, cw):
            """dst[128, cw+2] = src[:, c0-1 : c0+cw+1] * B[...], with edge guards."""
            lo, hi = c0 - 1, c0 + cw + 1
            dlo = 0
            if lo < 0:
                nc.vector.memset(dst[:, 0:1], 0.0)
                lo, dlo = 0, 1
            if hi > N:
                nc.vector.memset(dst[:, cw + 1:cw + 2], 0.0)
                hi = N
            nc.vector.tensor_mul(dst[:, dlo:dlo + (hi - lo)],
                                 src_t[:, src_off + lo:src_off + hi], Bt[:, lo:hi])

        # L1 aggregation: per supertile group, per head
        for (g0, gw) in SUP:
            po_h0 = psA.tile([HID, 1024], F32, tag="ps_h0")
            po_h1 = psB.tile([HID, 1024], F32, tag="ps_h1")
            po = {0: po_h0, 1: po_h1}
            Xp = xch.tile([128, 1026], BF16, tag="Xp")
            Xs = xch.tile([128, 1026], BF16, tag="Xs")
            Xn = xch.tile([128, 1026], BF16, tag="Xn")
            prescale(Xp, t_x, 0, B_p, g0, gw)
            prescale(Xs, t_x, 0, B_s, g0, gw)
            prescale(Xn, t_x, 0, B_n, g0, gw)
            for (c0, cw) in inner(g0, gw):
                o = c0 - g0
                for h in (0, 1):
                    lhs = t_w1blk[:, h * HID:(h + 1) * HID]
                    nc.tensor.matmul(po[h][:, o:o + cw], lhs, Xp[:, o:o + cw], start=True, stop=False)
                    nc.tensor.matmul(po[h][:, o:o + cw], lhs, Xs[:, o + 1:o + 1 + cw], start=False, stop=False)
                    nc.tensor.matmul(po[h][:, o:o + cw], lhs, Xn[:, o + 2:o + 2 + cw], start=False, stop=True)
            nc.scalar.activation(g_hi_t[g0][:, :gw], po[0][:, :gw], AF.Gelu)
            nc.scalar.activation(g_lo_t[g0][:, :gw], po[1][:, :gw], AF.Gelu)
            for (c0, cw) in inner(g0, gw):
                o = c0 - g0
                p_at = psC.tile([4, 512], F32, tag="atps")
                nc.tensor.matmul(p_at[0:2, :cw], t_waux2a[:], g_hi_t[g0][:, o:o + cw],
                                 start=True, stop=False)
                nc.tensor.matmul(p_at[0:2, :cw], t_waux2b[:], g_lo_t[g0][:, o:o + cw],
                                 start=False, stop=True)
                st = och.tile([4, 512], F32, tag="atstage")
                nc.vector.tensor_copy(st[0:2, :cw], p_at[0:2, :cw])
                nc.sync.dma_start(d_at[0:2, c0:c0 + cw], st[0:2, :cw])

        # ============ LAYER 2 ============
        h2 = big.tile([HID, N + 2], BF16, tag="xh")   # reuse x slot; guards at 0 and N+1
        nc.vector.memset(h2[:, 0:1], 0.0)
        nc.vector.memset(h2[:, N + 1:N + 2], 0.0)
        for (g0, gw) in SUP:
            po = psA.tile([HID, 1024], F32, tag="ps_h0")
            for (c0, cw) in inner(g0, gw):
                o = c0 - g0
                nc.tensor.matmul(po[:, o:o + cw], t_w2a[:], g_hi_t[g0][:, o:o + cw], start=True, stop=False)
                nc.tensor.matmul(po[:, o:o + cw], t_w2b[:], g_lo_t[g0][:, o:o + cw], start=False, stop=True)
            nc.scalar.copy(h2[:, 1 + g0:1 + g0 + gw], po[:, :gw])

        cm2_s = cmp_.tile([G, L], F32, tag="cm_s")
        cm2_d = cmp_.tile([G, L], F32, tag="cm_d")
        nc.sync.dma_start(cm2_s[:, :], d_at[0:1, :])
        nc.sync.dma_start(cm2_d[:, :], d_at[1:2, :])
        a2_p, a2_s, a2_n = chain_softmax(G, cm2_s, cm2_d)

        B2_p = big.tile([128, N], BF16, tag="B_p")
        B2_s = big.tile([128, N], BF16, tag="B_s")
        B2_n = big.tile([128, N], BF16, tag="B_n")
        make_B(a2_p, 1, 1, B2_p, 0)
        make_B(a2_s, 0, 1, B2_s, 1)
        make_B(a2_n, -1, 1, B2_n, 2)

        def prescale2(dst, src_t, src_off, Bt, c0, cw):
            lo, hi = c0 - 1, c0 + cw + 1
            dlo = 0
            if lo < 0:
                nc.vector.memset(dst[:, 0:1], 0.0)
                lo, dlo = 0, 1
            if hi > N:
                nc.vector.memset(dst[:, cw + 1:cw + 2], 0.0)
                hi = N
            n = hi - lo
            nc.vector.tensor_mul(dst[0:64, dlo:dlo + n],
                                 src_t[0:64, src_off + lo:src_off + hi], Bt[:, lo:hi])
            nc.vector.tensor_mul(dst[64:128, dlo:dlo + n],
                                 src_t[64:128, src_off + lo:src_off + hi], Bt[:, lo:hi])

        for (g0, gw) in SUP:
            po = psB.tile([HID, 1024], F32, tag="ps_h1")
            Hp = xch.tile([128, 1026], BF16, tag="Xp")
            Hs = xch.tile([128, 1026], BF16, tag="Xs")
            Hn = xch.tile([128, 1026], BF16, tag="Xn")
            prescale(Hp, h2, 1, B2_p, g0, gw)
            prescale(Hs, h2, 1, B2_s, g0, gw)
            prescale(Hn, h2, 1, B2_n, g0, gw)
            for (c0, cw) in inner(g0, gw):
                o = c0 - g0
                nc.tensor.matmul(po[:, o:o + cw], t_I[:], Hp[:, o:o + cw], start=True, stop=False)
                nc.tensor.matmul(po[:, o:o + cw], t_I[:], Hs[:, o + 1:o + 1 + cw], start=False, stop=False)
                nc.tensor.matmul(po[:, o:o + cw], t_I[:], Hn[:, o + 2:o + 2 + cw], start=False, stop=True)
            t_out = och.tile([HID, 1024], F32, tag="outch")
            nc.scalar.activation(t_out[:, :gw], po[:, :gw], AF.Gelu)
            nc.sync.dma_start(out[:, g0:g0 + gw], t_out[:, :gw])

    nc.compile()
    return nc


def _prep(inputs):
    x = np.asarray(inputs["x"], np.float32)
    W1 = np.asarray(inputs["W1"], np.float32)
    att_src1 = np.asarray(inputs["att_src1"], np.float32)
    att_dst1 = np.asarray(inputs["att_dst1"], np.float32)
    W2 = np.asarray(inputs["W2"], np.float32)
    att_src2 = np.asarray(inputs["att_src2"], np.float32)
    att_dst2 = np.asarray(inputs["att_dst2"], np.float32)

    x_pad = np.zeros((8 * N, D), np.float32)
    x_pad[:x.shape[0]] = x

    waux1 = np.stack([W1[:, 0:HID] @ att_src1[0], W1[:, HID:2 * HID] @ att_src1[1],
                      W1[:, 0:HID] @ att_dst1[0], W1[:, HID:2 * HID] @ att_dst1[1]], axis=1)
    w1blk = np.zeros((2 * D, 2 * HID), np.float32)
    w1blk[0:D, 0:HID] = W1[:, 0:HID]
    w1blk[D:2 * D, HID:2 * HID] = W1[:, HID:2 * HID]
    attv2 = np.stack([att_src2[0], att_dst2[0]], axis=1)
    waux2 = np.stack([W2 @ att_src2[0], W2 @ att_dst2[0]], axis=1)

    common = dict(
        waux1=waux1.astype(BF), w1blk=w1blk.astype(BF), w2=W2.astype(BF),
        attv2=attv2.astype(BF), waux2=waux2.astype(BF), ident=np.eye(HID, dtype=np.float32).astype(BF),
    )
    in_maps = []
    for c in range(8):
        xT = np.ascontiguousarray(x_pad[c * N:(c + 1) * N].T)
        in_maps.append(dict(common, xT2=np.vstack([xT, xT]).astype(BF)))
    return in_maps, x.shape[0]


def kernel(**inputs):
    if "nc" not in _cache:
        _cache["nc"] = build_nc()
    in_maps, n_real = _prep(inputs)
    res = run_bass_kernel_spmd(_cache["nc"], in_maps, core_ids=list(range(8)))
    outs = [res.results[c]["out"] for c in range(8)]
    full = np.concatenate([o.T for o in outs], axis=0)
    return np.ascontiguousarray(full[:n_real]).astype(np.float32)
